# revision 12
# baseline (speedup 1.0000x reference)
"""DCRNN Trainium2 kernel.

The reference module's diffusion convolution (supports/Wd/bd) and the r-gate
are dead code, so the live computation is a 2-layer GRU-style recurrence
applied independently to each of the B*N = 65536 (batch, node) tokens:

    for t in 0..11:
        u0 = sigmoid([x_t, h0] @ Wu0);  c0 = tanh([x_t, h0] @ Wc0)
        h0 = u0*h0 + (1-u0)*c0
        u1 = sigmoid([h0, h1] @ Wu1);   c1 = tanh([h0, h1] @ Wc1)
        h1 = u1*h1 + (1-u1)*c1
    out = h1 @ Wo + bo

Device formulation (per token, exact rewrite):
    tau = tanh(pre_u / 2)          -> u = (1+tau)/2, 1-u = (1-tau)/2
    c   = tanh(pre_c)
    h'  = a*h + b*c,  a = 0.5*tau+0.5, b = -0.5*tau+0.5

Data-parallel over tokens: 8 cores x 8192 tokens. On each core tokens are
split into G0 (SBUF partitions 0:64) and G1 (partitions 64:128) with
mirrored [tau|c] / [c|tau] column layouts so the gate multiply runs as a
single full-width 128-partition DVE op; the final a*h + b*c sum runs as a
gpsimd accumulate-DMA (or a DVE add, see USE_ACCUM_DMA).
"""

import numpy as np
import ml_dtypes

import concourse.bass as bass
import concourse.bacc as bacc
import concourse.mybir as mybir
import concourse.tile as tile
from concourse.bass_utils import run_bass_kernel_spmd

F16 = mybir.dt.float16
F32 = mybir.dt.float32

B, T, N, D, H, O = 32, 12, 2048, 2, 64, 1
NCORES = 8
TOK = (B * N) // NCORES          # tokens per core = 8192
G = TOK // 2                     # tokens per group = 4096
HALF = G // 2                    # elementwise phase free-dim = 2048
NMM = HALF // 512                # 512-wide matmuls per phase stream = 4

USE_ACCUM_DMA = False            # state sum via gpsimd accum DMA vs DVE add

_CACHE = {}


def _build_program():
    nc = bacc.Bacc("TRN2", target_bir_lowering=False, debug=False,
                   num_devices=NCORES)

    x_in = nc.dram_tensor("xin", [T, 2 * D, G], F16, kind="ExternalInput")
    w_x0 = nc.dram_tensor("wx0", [128, 128], F16, kind="ExternalInput")
    w_h0 = nc.dram_tensor("wh0", [128, 128], F16, kind="ExternalInput")
    w_a1 = nc.dram_tensor("wa1", [128, 128], F16, kind="ExternalInput")
    w_b1 = nc.dram_tensor("wb1", [128, 128], F16, kind="ExternalInput")
    w_o = nc.dram_tensor("wo", [128, 1], F16, kind="ExternalInput")
    b_in = nc.dram_tensor("bias", [128, 4], F32, kind="ExternalInput")
    # full gathered output on every core (AllGather below), so the host can
    # fetch a single device's shard instead of 8 per-core buffers
    out_d = nc.dram_tensor("out", [2 * NCORES, G], F32, kind="ExternalOutput")

    mm = nc.tensor.matmul
    TANH = mybir.ActivationFunctionType.Tanh
    COPY = mybir.ActivationFunctionType.Copy
    MULT = mybir.AluOpType.mult
    ADD = mybir.AluOpType.add

    with tile.TileContext(nc) as tc:
        with (
            tc.tile_pool(name="const", bufs=1) as const,
            tc.tile_pool(name="state", bufs=1) as state,

            tc.tile_pool(name="act", bufs=4) as actp,
            tc.tile_pool(name="ps", bufs=2, space="PSUM") as psp,
            tc.tile_pool(name="osb", bufs=1) as osbp,
            tc.tile_pool(name="dram", bufs=1, space="DRAM") as dramp,
        ):
            wx0 = const.tile([128, 128], F16, tag="wx0")
            wh0 = const.tile([128, 128], F16, tag="wh0")
            wa1 = const.tile([128, 128], F16, tag="wa1")
            wb1 = const.tile([128, 128], F16, tag="wb1")
            wo = const.tile([128, 1], F16, tag="wo")
            bia = const.tile([128, 4], F32, tag="bias")
            nc.sync.dma_start(wx0, w_x0[:, :])
            nc.sync.dma_start(wh0, w_h0[:, :])
            nc.sync.dma_start(wa1, w_a1[:, :])
            nc.sync.dma_start(wb1, w_b1[:, :])
            nc.sync.dma_start(wo, w_o[:, :])
            nc.sync.dma_start(bia, b_in[:, :])

            # states: S[l][g]; g=0 state rows 0:64 / scratch 64:128, g=1 mirrored
            S = [
                [state.tile([128, G], F16, tag=f"s{l}{g}", name=f"s{l}{g}") for g in (0, 1)]
                for l in (0, 1)
            ]
            for l in (0, 1):
                for g in (0, 1):
                    nc.vector.memset(S[l][g][:, :], 0.0)
            XT = [
                state.tile([128, G], F16, tag=f"xt{i}", name=f"xt{i}")
                for i in (0, 1)
            ]
            nc.vector.memset(XT[0][:, :], 0.0)
            nc.vector.memset(XT[1][:, :], 0.0)
            R = [
                [state.tile([128, G], F16, tag=f"r{l}{g}", name=f"r{l}{g}") for g in (0, 1)]
                for l in (0, 1)
            ]

            for t in range(T):
                xt = XT[t % 2]
                nc.sync.dma_start(xt[0:2, :], x_in[t, 0:2, :])
                nc.sync.dma_start(xt[64:66, :], x_in[t, 2:4, :])

                for l in (0, 1):
                    for hf in (0, 1):
                        sl = slice(hf * HALF, (hf + 1) * HALF)
                        ps = [psp.tile([128, HALF], F32, tag="ps", name="ps") for _ in (0, 1)]
                        # interleave G0/G1 matmuls -> different PE row groups
                        # overlap in the array
                        for k in range(NMM):
                            pc = slice(k * 512, (k + 1) * 512)
                            scol = slice(hf * HALF + k * 512, hf * HALF + (k + 1) * 512)
                            for g in (0, 1):
                                r0 = 64 * g
                                if l == 0:
                                    mm(
                                        ps[g][:, pc],
                                        wx0[r0 : r0 + 64, :],
                                        xt[r0 : r0 + 64, scol],
                                        start=True,
                                        stop=False,
                                    )
                                else:
                                    rs = slice(r0, r0 + 64)
                                    mm(
                                        ps[g][:, pc],
                                        wa1[rs, :],
                                        S[0][g][rs, scol],
                                        start=True,
                                        stop=False,
                                    )
                            for g in (0, 1):
                                r0 = 64 * g
                                rs = slice(r0, r0 + 64)
                                if l == 0:
                                    mm(
                                        ps[g][:, pc],
                                        wh0[rs, :],
                                        S[0][g][rs, scol],
                                        start=False,
                                        stop=True,
                                    )
                                else:
                                    mm(
                                        ps[g][:, pc],
                                        wb1[rs, :],
                                        S[1][g][rs, scol],
                                        start=False,
                                        stop=True,
                                    )
                        for g in (0, 1):
                            st = S[l][g]
                            a = actp.tile([128, HALF], F16, tag="act")
                            nc.scalar.activation(
                                a[:, :], ps[g][:, :], TANH, bias=bia[:, l * 2 + g : l * 2 + g + 1]
                            )
                            if g == 0:
                                tau, hrow, srow = a[0:64, :], slice(0, 64), slice(64, 128)
                            else:
                                tau, hrow, srow = a[64:128, :], slice(64, 128), slice(0, 64)
                            # b-gate into the scratch half of the state tensor
                            nc.vector.tensor_scalar(
                                st[srow, sl], tau, -0.5, 0.5, MULT, ADD
                            )
                            # tau -> a-gate in place
                            nc.vector.tensor_scalar(tau, tau, 0.5, 0.5, MULT, ADD)
                            # [a;c] (*) [h;b]  (G1: [c;a] (*) [b;h])
                            nc.vector.tensor_mul(st[:, sl], a[:, :], st[:, sl])
                    # state halves sum: h_new = a*h + b*c
                    for g in (0, 1):
                        st = S[l][g]
                        dst = slice(0, 64) if g == 0 else slice(64, 128)
                        srows = slice(64, 128) if g == 0 else slice(0, 64)
                        if USE_ACCUM_DMA:
                            nc.gpsimd.dma_start(
                                st[dst, :], st[srows, :], accum_op=ADD
                            )
                        else:
                            # realign the other product half to the same
                            # partition base via HWDGE DMA, then same-base add
                            rr = R[l][g]
                            nc.sync.dma_start(rr[dst, :], st[srows, :])
                            nc.vector.tensor_add(st[dst, :], st[dst, :], rr[dst, :])

            # output projection: out = h1 @ Wo  (bo added on host)
            osb = osbp.tile([128, G], F32, tag="osb")
            for hf in (0, 1):
                ps = [psp.tile([128, HALF], F32, tag="ps", name="ps") for _ in (0, 1)]
                for k in range(NMM):
                    pc = slice(k * 512, (k + 1) * 512)
                    scol = slice(hf * HALF + k * 512, hf * HALF + (k + 1) * 512)
                    mm(ps[0][0:1, pc], wo[0:64, :], S[1][0][0:64, scol],
                       start=True, stop=True)
                    mm(ps[1][64:65, pc], wo[64:128, :], S[1][1][64:128, scol],
                       start=True, stop=True)
                sl = slice(hf * HALF, (hf + 1) * HALF)
                nc.scalar.activation(osb[0:1, sl], ps[0][0:1, :], COPY)
                nc.scalar.activation(osb[64:65, sl], ps[1][64:65, :], COPY)
            # gather all cores' [2, G] into [2*NCORES, G] on every core:
            # collectives need DRAM bounce buffers (not I/O tensors)
            ob_in = dramp.tile([2, G], F32, tag="ob_in")
            ob_out = dramp.tile([2 * NCORES, G], F32, tag="ob_out")
            nc.sync.dma_start(ob_in[0:1, :], osb[0:1, :])
            nc.sync.dma_start(ob_in[1:2, :], osb[64:65, :])
            nc.gpsimd.collective_compute(
                "AllGather",
                mybir.AluOpType.bypass,
                replica_groups=[list(range(NCORES))],
                ins=[ob_in.opt()],
                outs=[ob_out.opt()],
            )
            nc.sync.dma_start(out_d[:, :], ob_out[:, :])

    nc.compile()
    return nc


def _fold_weights(Wu0, Wc0, Wu1, Wc1, Wo, bu0, bc0, bu1, bc1):
    """Host-side folding into the device layout (fp32 -> bf16)."""
    bf = np.float16

    def cell_w(Wu, Wc):  # [K, 64] x2 -> G0 [K,128] = [0.5*Wu | Wc], G1 swapped
        g0 = np.concatenate([0.5 * Wu, Wc], axis=1)
        g1 = np.concatenate([Wc, 0.5 * Wu], axis=1)
        return g0, g1

    def pack(g0, g1, k):
        w = np.zeros((128, 128), np.float32)
        w[0:k] = g0
        w[64 : 64 + k] = g1
        return w.astype(bf)

    wx0 = pack(*cell_w(Wu0[0:2], Wc0[0:2]), 2)
    wh0 = pack(*cell_w(Wu0[2:66], Wc0[2:66]), 64)
    wa1 = pack(*cell_w(Wu1[0:64], Wc1[0:64]), 64)
    wb1 = pack(*cell_w(Wu1[64:128], Wc1[64:128]), 64)
    wo = np.zeros((128, 1), np.float32)
    wo[0:64] = Wo
    wo[64:128] = Wo
    wo = wo.astype(bf)
    bias = np.zeros((128, 4), np.float32)
    for l, (bu, bc) in enumerate([(bu0, bc0), (bu1, bc1)]):
        bias[0:64, 2 * l + 0] = 0.5 * bu
        bias[64:128, 2 * l + 0] = bc
        bias[0:64, 2 * l + 1] = bc
        bias[64:128, 2 * l + 1] = 0.5 * bu
    return dict(wx0=wx0, wh0=wh0, wa1=wa1, wb1=wb1, wo=wo, bias=bias)


def _get_runner():
    """Build the Bass program once and wrap it in a persistent jitted
    shard_map callable (the hoisted equivalent of what bass_utils.
    run_bass_kernel_spmd -> bass2jax.run_bass_via_pjrt builds per call).

    run_bass_kernel_spmd constructs a fresh jax.jit closure on every
    invocation, which forces a full retrace + NEFF recompile (~450ms) per
    call; hoisting the jit out makes warm calls hit the C++ pjit fast path.
    """
    if "runner" in _CACHE:
        return _CACHE["runner"]

    import jax
    from jax.sharding import Mesh, PartitionSpec, NamedSharding
    from jax.experimental.shard_map import shard_map
    from concourse import bass2jax as b2j

    b2j.install_neuronx_cc_hook()
    nc = _build_program()
    assert nc.dbg_addr is None
    pname = nc.partition_id_tensor.name if nc.partition_id_tensor else None

    in_names, out_names, out_avals, zero_outs = [], [], [], []
    for alloc in nc.m.functions[0].allocations:
        if not isinstance(alloc, mybir.MemoryLocationSet):
            continue
        name = alloc.memorylocations[0].name
        if alloc.kind == "ExternalInput":
            if name != pname:
                in_names.append(name)
        elif alloc.kind == "ExternalOutput":
            out_names.append(name)
            shape = tuple(alloc.tensor_shape)
            dtype = mybir.dt.np(alloc.dtype)
            out_avals.append(jax.core.ShapedArray(shape, dtype))
            zero_outs.append(np.zeros((NCORES * shape[0], *shape[1:]), dtype))
    n_params = len(in_names)
    n_outs = len(out_names)
    all_names = tuple(in_names + out_names + ([pname] if pname else []))

    def _body(*args):
        operands = list(args)
        if pname is not None:
            operands.append(b2j.partition_id_tensor())
        return tuple(
            b2j._bass_exec_p.bind(
                *operands,
                out_avals=tuple(out_avals),
                in_names=all_names,
                out_names=tuple(out_names),
                lowering_input_output_aliases=(),
                sim_require_finite=True,
                sim_require_nnan=True,
                nc=nc,
            )
        )

    devices = jax.devices()[:NCORES]
    mesh = Mesh(np.asarray(devices), ("core",))
    specs = (PartitionSpec("core"),) * (n_params + n_outs)
    # No donate_argnums: the kernel writes every element of `out`, so the
    # zero-init aliasing run_bass_via_pjrt sets up is unnecessary; without
    # donation the zero operands can stay device-resident across calls.
    # out_specs is replicated: every core holds the full AllGathered output,
    # so the host fetches a single shard (1 RPC instead of 8).
    fn = jax.jit(
        shard_map(_body, mesh=mesh, in_specs=specs,
                  out_specs=(PartitionSpec(),) * n_outs, check_rep=False),
        keep_unused=True,
    )
    sharding = NamedSharding(mesh, PartitionSpec("core"))
    runner = dict(
        fn=fn, in_names=in_names,
        zero_dev=[jax.device_put(z, sharding) for z in zero_outs],
        sharding=sharding, jax=jax,
    )
    _CACHE["runner"] = runner
    return runner


def _dev_weights(inputs, runner):
    """Fold weights and park them on-device once (they are model constants);
    keyed by content so a different parameter set still recomputes."""
    names = ("Wu0", "Wc0", "Wu1", "Wc1", "Wo", "bu0", "bc0", "bu1", "bc1")
    ws = [np.asarray(inputs[n], np.float32) for n in names]
    key = hash(tuple(w.tobytes() for w in ws))
    if _CACHE.get("wkey") != key:
        folded = _fold_weights(*ws)
        jax_ = runner["jax"]
        _CACHE["wdev"] = {
            k: jax_.device_put(
                np.concatenate([v] * NCORES, axis=0), runner["sharding"]
            )
            for k, v in folded.items()
        }
        _CACHE["wkey"] = key
    return _CACHE["wdev"]


def kernel(**inputs):
    runner = _get_runner()
    wdev = _dev_weights(inputs, runner)
    bo = np.asarray(inputs["bo"], np.float32)

    # token order: flat (b, n); core c owns tokens [c*8192, (c+1)*8192)
    # per-core xin [T, 2D, G] -> global concat on axis 0: [NCORES*T, 2D, G].
    # xg[c*T+t, p*D+d, q*N+n] = x[4c+2p+q, t, n, d]; one strided copyto pass.
    x = np.asarray(inputs["x"], np.float32)
    xg = np.empty((NCORES * T, 2 * D, G), np.float16)
    np.copyto(
        xg.reshape(NCORES, T, 2, D, 2, N),
        x.reshape(NCORES, 2, 2, T, N, D).transpose(0, 3, 1, 5, 2, 4),
    )

    args = [xg if n == "xin" else wdev[n] for n in runner["in_names"]]
    out_arrs = runner["fn"](*args, *runner["zero_dev"])
    # replicated output: fetch exactly one device's [NCORES*2, G] buffer
    out = np.asarray(out_arrs[0].addressable_shards[0].data)
    return (out.reshape(-1).reshape(B, N, O) + bo).astype(np.float32)


if __name__ == "__main__":
    rng = np.random.default_rng(0)
    fake = {
        "x": rng.standard_normal((B, T, N, D), dtype=np.float32),
        "supports": rng.random((2, N, N), dtype=np.float32),
        "Wo": (rng.standard_normal((H, O)) * 0.02).astype(np.float32),
        "bo": np.zeros((O,), np.float32),
    }
    for l in range(2):
        din = (D if l == 0 else H) + H
        for g in ("r", "u", "c"):
            fake[f"W{g}{l}"] = (rng.standard_normal((din, H)) * 0.02).astype(np.float32)
            fake[f"b{g}{l}"] = np.zeros((H,), np.float32)
        fake[f"Wd{l}"] = (rng.standard_normal((2, H, H)) * 0.02).astype(np.float32)
        fake[f"bd{l}"] = np.zeros((2, H), np.float32)
    print(kernel(**fake).shape)



# revision 13
# speedup vs baseline: 1.2643x; 1.2643x over previous
"""DCRNN Trainium2 kernel.

The reference module's diffusion convolution (supports/Wd/bd) and the r-gate
are dead code, so the live computation is a 2-layer GRU-style recurrence
applied independently to each of the B*N = 65536 (batch, node) tokens:

    for t in 0..11:
        u0 = sigmoid([x_t, h0] @ Wu0);  c0 = tanh([x_t, h0] @ Wc0)
        h0 = u0*h0 + (1-u0)*c0
        u1 = sigmoid([h0, h1] @ Wu1);   c1 = tanh([h0, h1] @ Wc1)
        h1 = u1*h1 + (1-u1)*c1
    out = h1 @ Wo + bo

Device formulation (per token, exact rewrite):
    tau = tanh(pre_u / 2)          -> u = (1+tau)/2, 1-u = (1-tau)/2
    c   = tanh(pre_c)
    h'  = a*h + b*c,  a = 0.5*tau+0.5, b = -0.5*tau+0.5

Data-parallel over tokens: 8 cores x 8192 tokens. On each core tokens are
split into G0 (SBUF partitions 0:64) and G1 (partitions 64:128) with
mirrored [tau|c] / [c|tau] column layouts so the gate multiply runs as a
single full-width 128-partition DVE op; the final a*h + b*c sum runs as a
gpsimd accumulate-DMA (or a DVE add, see USE_ACCUM_DMA).
"""

import numpy as np
import ml_dtypes

import concourse.bass as bass
import concourse.bacc as bacc
import concourse.mybir as mybir
import concourse.tile as tile
from concourse.bass_utils import run_bass_kernel_spmd

F16 = mybir.dt.float16
F32 = mybir.dt.float32

B, T, N, D, H, O = 32, 12, 2048, 2, 64, 1
NCORES = 8
TOK = (B * N) // NCORES          # tokens per core = 8192
G = TOK // 2                     # tokens per group = 4096
HALF = G // 2                    # elementwise phase free-dim = 2048
NMM = HALF // 512                # 512-wide matmuls per phase stream = 4

USE_ACCUM_DMA = False            # state sum via gpsimd accum DMA vs DVE add

_CACHE = {}


def _build_program():
    nc = bacc.Bacc("TRN2", target_bir_lowering=False, debug=False,
                   num_devices=NCORES)

    x_in = nc.dram_tensor("xin", [T, 2 * D, G], F16, kind="ExternalInput")
    w_x0 = nc.dram_tensor("wx0", [128, 128], F16, kind="ExternalInput")
    w_h0 = nc.dram_tensor("wh0", [128, 128], F16, kind="ExternalInput")
    w_a1 = nc.dram_tensor("wa1", [128, 128], F16, kind="ExternalInput")
    w_b1 = nc.dram_tensor("wb1", [128, 128], F16, kind="ExternalInput")
    w_o = nc.dram_tensor("wo", [128, 1], F16, kind="ExternalInput")
    b_in = nc.dram_tensor("bias", [128, 4], F32, kind="ExternalInput")
    out_d = nc.dram_tensor("out", [2, G], F32, kind="ExternalOutput")

    mm = nc.tensor.matmul
    TANH = mybir.ActivationFunctionType.Tanh
    COPY = mybir.ActivationFunctionType.Copy
    MULT = mybir.AluOpType.mult
    ADD = mybir.AluOpType.add

    with tile.TileContext(nc) as tc:
        with (
            tc.tile_pool(name="const", bufs=1) as const,
            tc.tile_pool(name="state", bufs=1) as state,

            tc.tile_pool(name="act", bufs=4) as actp,
            tc.tile_pool(name="ps", bufs=2, space="PSUM") as psp,
            tc.tile_pool(name="osb", bufs=1) as osbp,
        ):
            wx0 = const.tile([128, 128], F16, tag="wx0")
            wh0 = const.tile([128, 128], F16, tag="wh0")
            wa1 = const.tile([128, 128], F16, tag="wa1")
            wb1 = const.tile([128, 128], F16, tag="wb1")
            wo = const.tile([128, 1], F16, tag="wo")
            bia = const.tile([128, 4], F32, tag="bias")
            nc.sync.dma_start(wx0, w_x0[:, :])
            nc.sync.dma_start(wh0, w_h0[:, :])
            nc.sync.dma_start(wa1, w_a1[:, :])
            nc.sync.dma_start(wb1, w_b1[:, :])
            nc.sync.dma_start(wo, w_o[:, :])
            nc.sync.dma_start(bia, b_in[:, :])

            # states: S[l][g]; g=0 state rows 0:64 / scratch 64:128, g=1 mirrored
            S = [
                [state.tile([128, G], F16, tag=f"s{l}{g}", name=f"s{l}{g}") for g in (0, 1)]
                for l in (0, 1)
            ]
            for l in (0, 1):
                for g in (0, 1):
                    nc.vector.memset(S[l][g][:, :], 0.0)
            XT = [
                state.tile([128, G], F16, tag=f"xt{i}", name=f"xt{i}")
                for i in (0, 1)
            ]
            nc.vector.memset(XT[0][:, :], 0.0)
            nc.vector.memset(XT[1][:, :], 0.0)
            R = [
                [state.tile([128, G], F16, tag=f"r{l}{g}", name=f"r{l}{g}") for g in (0, 1)]
                for l in (0, 1)
            ]

            for t in range(T):
                xt = XT[t % 2]
                nc.sync.dma_start(xt[0:2, :], x_in[t, 0:2, :])
                nc.sync.dma_start(xt[64:66, :], x_in[t, 2:4, :])

                for l in (0, 1):
                    for hf in (0, 1):
                        sl = slice(hf * HALF, (hf + 1) * HALF)
                        ps = [psp.tile([128, HALF], F32, tag="ps", name="ps") for _ in (0, 1)]
                        # interleave G0/G1 matmuls -> different PE row groups
                        # overlap in the array
                        for k in range(NMM):
                            pc = slice(k * 512, (k + 1) * 512)
                            scol = slice(hf * HALF + k * 512, hf * HALF + (k + 1) * 512)
                            for g in (0, 1):
                                r0 = 64 * g
                                if l == 0:
                                    mm(
                                        ps[g][:, pc],
                                        wx0[r0 : r0 + 64, :],
                                        xt[r0 : r0 + 64, scol],
                                        start=True,
                                        stop=False,
                                    )
                                else:
                                    rs = slice(r0, r0 + 64)
                                    mm(
                                        ps[g][:, pc],
                                        wa1[rs, :],
                                        S[0][g][rs, scol],
                                        start=True,
                                        stop=False,
                                    )
                            for g in (0, 1):
                                r0 = 64 * g
                                rs = slice(r0, r0 + 64)
                                if l == 0:
                                    mm(
                                        ps[g][:, pc],
                                        wh0[rs, :],
                                        S[0][g][rs, scol],
                                        start=False,
                                        stop=True,
                                    )
                                else:
                                    mm(
                                        ps[g][:, pc],
                                        wb1[rs, :],
                                        S[1][g][rs, scol],
                                        start=False,
                                        stop=True,
                                    )
                        for g in (0, 1):
                            st = S[l][g]
                            a = actp.tile([128, HALF], F16, tag="act")
                            nc.scalar.activation(
                                a[:, :], ps[g][:, :], TANH, bias=bia[:, l * 2 + g : l * 2 + g + 1]
                            )
                            if g == 0:
                                tau, hrow, srow = a[0:64, :], slice(0, 64), slice(64, 128)
                            else:
                                tau, hrow, srow = a[64:128, :], slice(64, 128), slice(0, 64)
                            # b-gate into the scratch half of the state tensor
                            nc.vector.tensor_scalar(
                                st[srow, sl], tau, -0.5, 0.5, MULT, ADD
                            )
                            # tau -> a-gate in place
                            nc.vector.tensor_scalar(tau, tau, 0.5, 0.5, MULT, ADD)
                            # [a;c] (*) [h;b]  (G1: [c;a] (*) [b;h])
                            nc.vector.tensor_mul(st[:, sl], a[:, :], st[:, sl])
                    # state halves sum: h_new = a*h + b*c
                    for g in (0, 1):
                        st = S[l][g]
                        dst = slice(0, 64) if g == 0 else slice(64, 128)
                        srows = slice(64, 128) if g == 0 else slice(0, 64)
                        if USE_ACCUM_DMA:
                            nc.gpsimd.dma_start(
                                st[dst, :], st[srows, :], accum_op=ADD
                            )
                        else:
                            # realign the other product half to the same
                            # partition base via HWDGE DMA, then same-base add
                            rr = R[l][g]
                            nc.sync.dma_start(rr[dst, :], st[srows, :])
                            nc.vector.tensor_add(st[dst, :], st[dst, :], rr[dst, :])

            # output projection: out = h1 @ Wo  (bo added on host)
            osb = osbp.tile([128, G], F32, tag="osb")
            for hf in (0, 1):
                ps = [psp.tile([128, HALF], F32, tag="ps", name="ps") for _ in (0, 1)]
                for k in range(NMM):
                    pc = slice(k * 512, (k + 1) * 512)
                    scol = slice(hf * HALF + k * 512, hf * HALF + (k + 1) * 512)
                    mm(ps[0][0:1, pc], wo[0:64, :], S[1][0][0:64, scol],
                       start=True, stop=True)
                    mm(ps[1][64:65, pc], wo[64:128, :], S[1][1][64:128, scol],
                       start=True, stop=True)
                sl = slice(hf * HALF, (hf + 1) * HALF)
                nc.scalar.activation(osb[0:1, sl], ps[0][0:1, :], COPY)
                nc.scalar.activation(osb[64:65, sl], ps[1][64:65, :], COPY)
            nc.sync.dma_start(out_d[0:1, :], osb[0:1, :])
            nc.sync.dma_start(out_d[1:2, :], osb[64:65, :])

    nc.compile()
    return nc


def _fold_weights(Wu0, Wc0, Wu1, Wc1, Wo, bu0, bc0, bu1, bc1):
    """Host-side folding into the device layout (fp32 -> bf16)."""
    bf = np.float16

    def cell_w(Wu, Wc):  # [K, 64] x2 -> G0 [K,128] = [0.5*Wu | Wc], G1 swapped
        g0 = np.concatenate([0.5 * Wu, Wc], axis=1)
        g1 = np.concatenate([Wc, 0.5 * Wu], axis=1)
        return g0, g1

    def pack(g0, g1, k):
        w = np.zeros((128, 128), np.float32)
        w[0:k] = g0
        w[64 : 64 + k] = g1
        return w.astype(bf)

    wx0 = pack(*cell_w(Wu0[0:2], Wc0[0:2]), 2)
    wh0 = pack(*cell_w(Wu0[2:66], Wc0[2:66]), 64)
    wa1 = pack(*cell_w(Wu1[0:64], Wc1[0:64]), 64)
    wb1 = pack(*cell_w(Wu1[64:128], Wc1[64:128]), 64)
    wo = np.zeros((128, 1), np.float32)
    wo[0:64] = Wo
    wo[64:128] = Wo
    wo = wo.astype(bf)
    bias = np.zeros((128, 4), np.float32)
    for l, (bu, bc) in enumerate([(bu0, bc0), (bu1, bc1)]):
        bias[0:64, 2 * l + 0] = 0.5 * bu
        bias[64:128, 2 * l + 0] = bc
        bias[0:64, 2 * l + 1] = bc
        bias[64:128, 2 * l + 1] = 0.5 * bu
    return dict(wx0=wx0, wh0=wh0, wa1=wa1, wb1=wb1, wo=wo, bias=bias)


def _get_runner():
    """Build the Bass program once and wrap it in a persistent jitted
    shard_map callable (the hoisted equivalent of what bass_utils.
    run_bass_kernel_spmd -> bass2jax.run_bass_via_pjrt builds per call).

    run_bass_kernel_spmd constructs a fresh jax.jit closure on every
    invocation, which forces a full retrace + NEFF recompile (~450ms) per
    call; hoisting the jit out makes warm calls hit the C++ pjit fast path.
    """
    if "runner" in _CACHE:
        return _CACHE["runner"]

    import jax
    from jax.sharding import Mesh, PartitionSpec, NamedSharding
    from jax.experimental.shard_map import shard_map
    from concourse import bass2jax as b2j

    b2j.install_neuronx_cc_hook()
    nc = _build_program()
    assert nc.dbg_addr is None
    pname = nc.partition_id_tensor.name if nc.partition_id_tensor else None

    in_names, out_names, out_avals, zero_outs = [], [], [], []
    for alloc in nc.m.functions[0].allocations:
        if not isinstance(alloc, mybir.MemoryLocationSet):
            continue
        name = alloc.memorylocations[0].name
        if alloc.kind == "ExternalInput":
            if name != pname:
                in_names.append(name)
        elif alloc.kind == "ExternalOutput":
            out_names.append(name)
            shape = tuple(alloc.tensor_shape)
            dtype = mybir.dt.np(alloc.dtype)
            out_avals.append(jax.core.ShapedArray(shape, dtype))
            zero_outs.append(np.zeros((NCORES * shape[0], *shape[1:]), dtype))
    n_params = len(in_names)
    n_outs = len(out_names)
    all_names = tuple(in_names + out_names + ([pname] if pname else []))

    def _body(*args):
        operands = list(args)
        if pname is not None:
            operands.append(b2j.partition_id_tensor())
        return tuple(
            b2j._bass_exec_p.bind(
                *operands,
                out_avals=tuple(out_avals),
                in_names=all_names,
                out_names=tuple(out_names),
                lowering_input_output_aliases=(),
                sim_require_finite=True,
                sim_require_nnan=True,
                nc=nc,
            )
        )

    devices = jax.devices()[:NCORES]
    mesh = Mesh(np.asarray(devices), ("core",))
    specs = (PartitionSpec("core"),) * (n_params + n_outs)
    # No donate_argnums: the kernel writes every element of `out`, so the
    # zero-init aliasing run_bass_via_pjrt sets up is unnecessary; without
    # donation the zero operands can stay device-resident across calls.
    fn = jax.jit(
        shard_map(_body, mesh=mesh, in_specs=specs, out_specs=specs[:n_outs],
                  check_rep=False),
        keep_unused=True,
    )
    sharding = NamedSharding(mesh, PartitionSpec("core"))
    runner = dict(
        fn=fn, in_names=in_names,
        zero_dev=[jax.device_put(z, sharding) for z in zero_outs],
        sharding=sharding, jax=jax,
    )
    _CACHE["runner"] = runner
    return runner


def _dev_weights(inputs, runner):
    """Fold weights and park them on-device once (they are model constants);
    keyed by content so a different parameter set still recomputes."""
    names = ("Wu0", "Wc0", "Wu1", "Wc1", "Wo", "bu0", "bc0", "bu1", "bc1")
    ws = [np.asarray(inputs[n], np.float32) for n in names]
    key = hash(tuple(w.tobytes() for w in ws))
    if _CACHE.get("wkey") != key:
        folded = _fold_weights(*ws)
        jax_ = runner["jax"]
        _CACHE["wdev"] = {
            k: jax_.device_put(
                np.concatenate([v] * NCORES, axis=0), runner["sharding"]
            )
            for k, v in folded.items()
        }
        _CACHE["wkey"] = key
    return _CACHE["wdev"]


def kernel(**inputs):
    runner = _get_runner()
    wdev = _dev_weights(inputs, runner)
    bo = np.asarray(inputs["bo"], np.float32)

    # token order: flat (b, n); core c owns tokens [c*8192, (c+1)*8192)
    # per-core xin [T, 2D, G] -> global concat on axis 0: [NCORES*T, 2D, G].
    # xg[c*T+t, p*D+d, q*N+n] = x[4c+2p+q, t, n, d]; one strided copyto pass.
    x = np.asarray(inputs["x"], np.float32)
    xg = np.empty((NCORES * T, 2 * D, G), np.float16)
    np.copyto(
        xg.reshape(NCORES, T, 2, D, 2, N),
        x.reshape(NCORES, 2, 2, T, N, D).transpose(0, 3, 1, 5, 2, 4),
    )

    args = [xg if n == "xin" else wdev[n] for n in runner["in_names"]]
    out_arrs = runner["fn"](*args, *runner["zero_dev"])
    out = np.asarray(out_arrs[0])  # [NCORES*2, G] f32, one blocking fetch
    return (out.reshape(-1).reshape(B, N, O) + bo).astype(np.float32)


if __name__ == "__main__":
    rng = np.random.default_rng(0)
    fake = {
        "x": rng.standard_normal((B, T, N, D), dtype=np.float32),
        "supports": rng.random((2, N, N), dtype=np.float32),
        "Wo": (rng.standard_normal((H, O)) * 0.02).astype(np.float32),
        "bo": np.zeros((O,), np.float32),
    }
    for l in range(2):
        din = (D if l == 0 else H) + H
        for g in ("r", "u", "c"):
            fake[f"W{g}{l}"] = (rng.standard_normal((din, H)) * 0.02).astype(np.float32)
            fake[f"b{g}{l}"] = np.zeros((H,), np.float32)
        fake[f"Wd{l}"] = (rng.standard_normal((2, H, H)) * 0.02).astype(np.float32)
        fake[f"bd{l}"] = np.zeros((2, H), np.float32)
    print(kernel(**fake).shape)



# revision 16
# speedup vs baseline: 1.3868x; 1.0969x over previous
"""DCRNN Trainium2 kernel.

The reference module's diffusion convolution (supports/Wd/bd) and the r-gate
are dead code, so the live computation is a 2-layer GRU-style recurrence
applied independently to each of the B*N = 65536 (batch, node) tokens:

    for t in 0..11:
        u0 = sigmoid([x_t, h0] @ Wu0);  c0 = tanh([x_t, h0] @ Wc0)
        h0 = u0*h0 + (1-u0)*c0
        u1 = sigmoid([h0, h1] @ Wu1);   c1 = tanh([h0, h1] @ Wc1)
        h1 = u1*h1 + (1-u1)*c1
    out = h1 @ Wo + bo

Device formulation (per token, exact rewrite):
    tau = tanh(pre_u / 2)          -> u = (1+tau)/2, 1-u = (1-tau)/2
    c   = tanh(pre_c)
    h'  = a*h + b*c,  a = 0.5*tau+0.5, b = -0.5*tau+0.5

Data-parallel over tokens: 8 cores x 8192 tokens. On each core tokens are
split into G0 (SBUF partitions 0:64) and G1 (partitions 64:128) with
mirrored [tau|c] / [c|tau] column layouts so the gate multiply runs as a
single full-width 128-partition DVE op; the final a*h + b*c sum runs as a
gpsimd accumulate-DMA (or a DVE add, see USE_ACCUM_DMA).
"""

import numpy as np
import ml_dtypes

import concourse.bass as bass
import concourse.bacc as bacc
import concourse.mybir as mybir
import concourse.tile as tile
from concourse.bass_utils import run_bass_kernel_spmd

F16 = mybir.dt.float16
F32 = mybir.dt.float32

B, T, N, D, H, O = 32, 12, 2048, 2, 64, 1
NCORES = 8
TOK = (B * N) // NCORES          # tokens per core = 8192
G = TOK // 2                     # tokens per group = 4096
HALF = G // 2                    # elementwise phase free-dim = 2048
NMM = HALF // 512                # 512-wide matmuls per phase stream = 4

USE_ACCUM_DMA = False            # state sum via gpsimd accum DMA vs DVE add

_CACHE = {}


def _build_program():
    nc = bacc.Bacc("TRN2", target_bir_lowering=False, debug=False,
                   num_devices=NCORES)

    x_in = nc.dram_tensor("xin", [T, 2 * D, G], F16, kind="ExternalInput")
    w_x0 = nc.dram_tensor("wx0", [128, 128], F16, kind="ExternalInput")
    w_h0 = nc.dram_tensor("wh0", [128, 128], F16, kind="ExternalInput")
    w_a1 = nc.dram_tensor("wa1", [128, 128], F16, kind="ExternalInput")
    w_b1 = nc.dram_tensor("wb1", [128, 128], F16, kind="ExternalInput")
    w_o = nc.dram_tensor("wo", [128, 1], F16, kind="ExternalInput")
    b_in = nc.dram_tensor("bias", [128, 4], F32, kind="ExternalInput")
    out_d = nc.dram_tensor("out", [2, G], F16, kind="ExternalOutput")

    mm = nc.tensor.matmul
    TANH = mybir.ActivationFunctionType.Tanh
    COPY = mybir.ActivationFunctionType.Copy
    MULT = mybir.AluOpType.mult
    ADD = mybir.AluOpType.add

    with tile.TileContext(nc) as tc:
        with (
            tc.tile_pool(name="const", bufs=1) as const,
            tc.tile_pool(name="state", bufs=1) as state,

            tc.tile_pool(name="act", bufs=4) as actp,
            tc.tile_pool(name="ps", bufs=2, space="PSUM") as psp,
            tc.tile_pool(name="osb", bufs=1) as osbp,
        ):
            wx0 = const.tile([128, 128], F16, tag="wx0")
            wh0 = const.tile([128, 128], F16, tag="wh0")
            wa1 = const.tile([128, 128], F16, tag="wa1")
            wb1 = const.tile([128, 128], F16, tag="wb1")
            wo = const.tile([128, 1], F16, tag="wo")
            bia = const.tile([128, 4], F32, tag="bias")
            nc.sync.dma_start(wx0, w_x0[:, :])
            nc.sync.dma_start(wh0, w_h0[:, :])
            nc.sync.dma_start(wa1, w_a1[:, :])
            nc.sync.dma_start(wb1, w_b1[:, :])
            nc.sync.dma_start(wo, w_o[:, :])
            nc.sync.dma_start(bia, b_in[:, :])

            # states: S[l][g]; g=0 state rows 0:64 / scratch 64:128, g=1 mirrored
            S = [
                [state.tile([128, G], F16, tag=f"s{l}{g}", name=f"s{l}{g}") for g in (0, 1)]
                for l in (0, 1)
            ]
            for l in (0, 1):
                for g in (0, 1):
                    nc.vector.memset(S[l][g][:, :], 0.0)
            XT = [
                state.tile([128, G], F16, tag=f"xt{i}", name=f"xt{i}")
                for i in (0, 1)
            ]
            nc.vector.memset(XT[0][:, :], 0.0)
            nc.vector.memset(XT[1][:, :], 0.0)
            R = [
                [state.tile([128, G], F16, tag=f"r{l}{g}", name=f"r{l}{g}") for g in (0, 1)]
                for l in (0, 1)
            ]

            for t in range(T):
                xt = XT[t % 2]
                nc.sync.dma_start(xt[0:2, :], x_in[t, 0:2, :])
                nc.sync.dma_start(xt[64:66, :], x_in[t, 2:4, :])

                for l in (0, 1):
                    for hf in (0, 1):
                        sl = slice(hf * HALF, (hf + 1) * HALF)
                        ps = [psp.tile([128, HALF], F32, tag="ps", name="ps") for _ in (0, 1)]
                        # interleave G0/G1 matmuls -> different PE row groups
                        # overlap in the array
                        for k in range(NMM):
                            pc = slice(k * 512, (k + 1) * 512)
                            scol = slice(hf * HALF + k * 512, hf * HALF + (k + 1) * 512)
                            for g in (0, 1):
                                r0 = 64 * g
                                if l == 0:
                                    mm(
                                        ps[g][:, pc],
                                        wx0[r0 : r0 + 64, :],
                                        xt[r0 : r0 + 64, scol],
                                        start=True,
                                        stop=False,
                                    )
                                else:
                                    rs = slice(r0, r0 + 64)
                                    mm(
                                        ps[g][:, pc],
                                        wa1[rs, :],
                                        S[0][g][rs, scol],
                                        start=True,
                                        stop=False,
                                    )
                            for g in (0, 1):
                                r0 = 64 * g
                                rs = slice(r0, r0 + 64)
                                if l == 0:
                                    mm(
                                        ps[g][:, pc],
                                        wh0[rs, :],
                                        S[0][g][rs, scol],
                                        start=False,
                                        stop=True,
                                    )
                                else:
                                    mm(
                                        ps[g][:, pc],
                                        wb1[rs, :],
                                        S[1][g][rs, scol],
                                        start=False,
                                        stop=True,
                                    )
                        for g in (0, 1):
                            st = S[l][g]
                            a = actp.tile([128, HALF], F16, tag="act")
                            nc.scalar.activation(
                                a[:, :], ps[g][:, :], TANH, bias=bia[:, l * 2 + g : l * 2 + g + 1]
                            )
                            if g == 0:
                                tau, hrow, srow = a[0:64, :], slice(0, 64), slice(64, 128)
                            else:
                                tau, hrow, srow = a[64:128, :], slice(64, 128), slice(0, 64)
                            # b-gate into the scratch half of the state tensor
                            nc.vector.tensor_scalar(
                                st[srow, sl], tau, -0.5, 0.5, MULT, ADD
                            )
                            # tau -> a-gate in place
                            nc.vector.tensor_scalar(tau, tau, 0.5, 0.5, MULT, ADD)
                            # [a;c] (*) [h;b]  (G1: [c;a] (*) [b;h])
                            nc.vector.tensor_mul(st[:, sl], a[:, :], st[:, sl])
                    # state halves sum: h_new = a*h + b*c
                    for g in (0, 1):
                        st = S[l][g]
                        dst = slice(0, 64) if g == 0 else slice(64, 128)
                        srows = slice(64, 128) if g == 0 else slice(0, 64)
                        if USE_ACCUM_DMA:
                            nc.gpsimd.dma_start(
                                st[dst, :], st[srows, :], accum_op=ADD
                            )
                        else:
                            # realign the other product half to the same
                            # partition base via HWDGE DMA, then same-base add
                            rr = R[l][g]
                            nc.sync.dma_start(rr[dst, :], st[srows, :])
                            nc.vector.tensor_add(st[dst, :], st[dst, :], rr[dst, :])

            # output projection: out = h1 @ Wo  (bo added on host)
            osb = osbp.tile([128, G], F16, tag="osb")
            for hf in (0, 1):
                ps = [psp.tile([128, HALF], F32, tag="ps", name="ps") for _ in (0, 1)]
                for k in range(NMM):
                    pc = slice(k * 512, (k + 1) * 512)
                    scol = slice(hf * HALF + k * 512, hf * HALF + (k + 1) * 512)
                    mm(ps[0][0:1, pc], wo[0:64, :], S[1][0][0:64, scol],
                       start=True, stop=True)
                    mm(ps[1][64:65, pc], wo[64:128, :], S[1][1][64:128, scol],
                       start=True, stop=True)
                sl = slice(hf * HALF, (hf + 1) * HALF)
                nc.scalar.activation(osb[0:1, sl], ps[0][0:1, :], COPY)
                nc.scalar.activation(osb[64:65, sl], ps[1][64:65, :], COPY)
            nc.sync.dma_start(out_d[0:1, :], osb[0:1, :])
            nc.sync.dma_start(out_d[1:2, :], osb[64:65, :])

    nc.compile()
    return nc


def _fold_weights(Wu0, Wc0, Wu1, Wc1, Wo, bu0, bc0, bu1, bc1):
    """Host-side folding into the device layout (fp32 -> bf16)."""
    bf = np.float16

    def cell_w(Wu, Wc):  # [K, 64] x2 -> G0 [K,128] = [0.5*Wu | Wc], G1 swapped
        g0 = np.concatenate([0.5 * Wu, Wc], axis=1)
        g1 = np.concatenate([Wc, 0.5 * Wu], axis=1)
        return g0, g1

    def pack(g0, g1, k):
        w = np.zeros((128, 128), np.float32)
        w[0:k] = g0
        w[64 : 64 + k] = g1
        return w.astype(bf)

    wx0 = pack(*cell_w(Wu0[0:2], Wc0[0:2]), 2)
    wh0 = pack(*cell_w(Wu0[2:66], Wc0[2:66]), 64)
    wa1 = pack(*cell_w(Wu1[0:64], Wc1[0:64]), 64)
    wb1 = pack(*cell_w(Wu1[64:128], Wc1[64:128]), 64)
    wo = np.zeros((128, 1), np.float32)
    wo[0:64] = Wo
    wo[64:128] = Wo
    wo = wo.astype(bf)
    bias = np.zeros((128, 4), np.float32)
    for l, (bu, bc) in enumerate([(bu0, bc0), (bu1, bc1)]):
        bias[0:64, 2 * l + 0] = 0.5 * bu
        bias[64:128, 2 * l + 0] = bc
        bias[0:64, 2 * l + 1] = bc
        bias[64:128, 2 * l + 1] = 0.5 * bu
    return dict(wx0=wx0, wh0=wh0, wa1=wa1, wb1=wb1, wo=wo, bias=bias)


def _get_runner():
    """Build the Bass program once and wrap it in a persistent jitted
    shard_map callable (the hoisted equivalent of what bass_utils.
    run_bass_kernel_spmd -> bass2jax.run_bass_via_pjrt builds per call).

    run_bass_kernel_spmd constructs a fresh jax.jit closure on every
    invocation, which forces a full retrace + NEFF recompile (~450ms) per
    call; hoisting the jit out makes warm calls hit the C++ pjit fast path.
    """
    if "runner" in _CACHE:
        return _CACHE["runner"]

    import jax
    from jax.sharding import Mesh, PartitionSpec, NamedSharding
    from jax.experimental.shard_map import shard_map
    from concourse import bass2jax as b2j

    b2j.install_neuronx_cc_hook()
    nc = _build_program()
    assert nc.dbg_addr is None
    pname = nc.partition_id_tensor.name if nc.partition_id_tensor else None

    in_names, out_names, out_avals, zero_outs = [], [], [], []
    for alloc in nc.m.functions[0].allocations:
        if not isinstance(alloc, mybir.MemoryLocationSet):
            continue
        name = alloc.memorylocations[0].name
        if alloc.kind == "ExternalInput":
            if name != pname:
                in_names.append(name)
        elif alloc.kind == "ExternalOutput":
            out_names.append(name)
            shape = tuple(alloc.tensor_shape)
            dtype = mybir.dt.np(alloc.dtype)
            out_avals.append(jax.core.ShapedArray(shape, dtype))
            zero_outs.append(np.zeros((NCORES * shape[0], *shape[1:]), dtype))
    n_params = len(in_names)
    n_outs = len(out_names)
    all_names = tuple(in_names + out_names + ([pname] if pname else []))

    def _body(*args):
        operands = list(args)
        if pname is not None:
            operands.append(b2j.partition_id_tensor())
        return tuple(
            b2j._bass_exec_p.bind(
                *operands,
                out_avals=tuple(out_avals),
                in_names=all_names,
                out_names=tuple(out_names),
                lowering_input_output_aliases=(),
                sim_require_finite=True,
                sim_require_nnan=True,
                nc=nc,
            )
        )

    devices = jax.devices()[:NCORES]
    mesh = Mesh(np.asarray(devices), ("core",))
    specs = (PartitionSpec("core"),) * (n_params + n_outs)
    # No donate_argnums: the kernel writes every element of `out`, so the
    # zero-init aliasing run_bass_via_pjrt sets up is unnecessary; without
    # donation the zero operands can stay device-resident across calls.
    fn = jax.jit(
        shard_map(_body, mesh=mesh, in_specs=specs, out_specs=specs[:n_outs],
                  check_rep=False),
        keep_unused=True,
    )
    sharding = NamedSharding(mesh, PartitionSpec("core"))
    runner = dict(
        fn=fn, in_names=in_names,
        zero_dev=[jax.device_put(z, sharding) for z in zero_outs],
        sharding=sharding, jax=jax,
    )
    _CACHE["runner"] = runner
    return runner


def _dev_weights(inputs, runner):
    """Fold weights and park them on-device once (they are model constants);
    keyed by content so a different parameter set still recomputes."""
    names = ("Wu0", "Wc0", "Wu1", "Wc1", "Wo", "bu0", "bc0", "bu1", "bc1")
    ws = [np.asarray(inputs[n], np.float32) for n in names]
    key = hash(tuple(w.tobytes() for w in ws))
    if _CACHE.get("wkey") != key:
        folded = _fold_weights(*ws)
        jax_ = runner["jax"]
        _CACHE["wdev"] = {
            k: jax_.device_put(
                np.concatenate([v] * NCORES, axis=0), runner["sharding"]
            )
            for k, v in folded.items()
        }
        _CACHE["wkey"] = key
    return _CACHE["wdev"]


def kernel(**inputs):
    runner = _get_runner()
    wdev = _dev_weights(inputs, runner)
    bo = np.asarray(inputs["bo"], np.float32)

    # token order: flat (b, n); core c owns tokens [c*8192, (c+1)*8192)
    # per-core xin [T, 2D, G] -> global concat on axis 0: [NCORES*T, 2D, G].
    # xg[c*T+t, p*D+d, q*N+n] = x[4c+2p+q, t, n, d]; one strided copyto pass
    # into a persistent buffer (avoids per-call page-fault cost).
    x = np.asarray(inputs["x"], np.float32)
    if "xg" not in _CACHE:
        _CACHE["xg"] = np.empty((NCORES * T, 2 * D, G), np.float16)
    xg = _CACHE["xg"]
    np.copyto(
        xg.reshape(NCORES, T, 2, D, 2, N),
        x.reshape(NCORES, 2, 2, T, N, D).transpose(0, 3, 1, 5, 2, 4),
    )

    args = [xg if n == "xin" else wdev[n] for n in runner["in_names"]]
    out_arrs = runner["fn"](*args, *runner["zero_dev"])
    out = np.asarray(out_arrs[0])  # [NCORES*2, G] f16, one blocking fetch
    return (out.reshape(-1).astype(np.float32).reshape(B, N, O) + bo)


if __name__ == "__main__":
    rng = np.random.default_rng(0)
    fake = {
        "x": rng.standard_normal((B, T, N, D), dtype=np.float32),
        "supports": rng.random((2, N, N), dtype=np.float32),
        "Wo": (rng.standard_normal((H, O)) * 0.02).astype(np.float32),
        "bo": np.zeros((O,), np.float32),
    }
    for l in range(2):
        din = (D if l == 0 else H) + H
        for g in ("r", "u", "c"):
            fake[f"W{g}{l}"] = (rng.standard_normal((din, H)) * 0.02).astype(np.float32)
            fake[f"b{g}{l}"] = np.zeros((H,), np.float32)
        fake[f"Wd{l}"] = (rng.standard_normal((2, H, H)) * 0.02).astype(np.float32)
        fake[f"bd{l}"] = np.zeros((2, H), np.float32)
    print(kernel(**fake).shape)



# revision 21
# speedup vs baseline: 1.5378x; 1.1089x over previous
"""DCRNN Trainium2 kernel.

The reference module's diffusion convolution (supports/Wd/bd) and the r-gate
are dead code, so the live computation is a 2-layer GRU-style recurrence
applied independently to each of the B*N = 65536 (batch, node) tokens:

    for t in 0..11:
        u0 = sigmoid([x_t, h0] @ Wu0);  c0 = tanh([x_t, h0] @ Wc0)
        h0 = u0*h0 + (1-u0)*c0
        u1 = sigmoid([h0, h1] @ Wu1);   c1 = tanh([h0, h1] @ Wc1)
        h1 = u1*h1 + (1-u1)*c1
    out = h1 @ Wo + bo

Device formulation (per token, exact rewrite):
    tau = tanh(pre_u / 2)          -> u = (1+tau)/2, 1-u = (1-tau)/2
    c   = tanh(pre_c)
    h'  = a*h + b*c,  a = 0.5*tau+0.5, b = -0.5*tau+0.5

Data-parallel over tokens: 8 cores x 8192 tokens. On each core tokens are
split into G0 (SBUF partitions 0:64) and G1 (partitions 64:128) with
mirrored [tau|c] / [c|tau] column layouts so the gate multiply runs as a
single full-width 128-partition DVE op; the final a*h + b*c sum runs as a
gpsimd accumulate-DMA (or a DVE add, see USE_ACCUM_DMA).

Dispatch: the per-call wall time is dominated by the axon tunnel (~69ms
RTT for any blocking round trip, ~150MB/s wire). The Bass program is
compiled once into a persistent jax.jit(shard_map) callable (the hoisted
equivalent of bass_utils.run_bass_kernel_spmd's axon path, which would
otherwise rebuild + recompile the NEFF on every call). Folded weights and
the unused zero-output operands are parked on-device; a warm call ships
only x (fp16, 3MB) up and the fp16 output (128KB) down in one async
dispatch chain with a single blocking fetch.
"""

import numpy as np
import ml_dtypes

import concourse.bass as bass
import concourse.bacc as bacc
import concourse.mybir as mybir
import concourse.tile as tile
from concourse.bass_utils import run_bass_kernel_spmd

F16 = mybir.dt.float16
F32 = mybir.dt.float32

B, T, N, D, H, O = 32, 12, 2048, 2, 64, 1
NCORES = 8
TOK = (B * N) // NCORES          # tokens per core = 8192
G = TOK // 2                     # tokens per group = 4096
HALF = G // 2                    # elementwise phase free-dim = 2048
NMM = HALF // 512                # 512-wide matmuls per phase stream = 4

USE_ACCUM_DMA = False            # state sum via gpsimd accum DMA vs DVE add

_CACHE = {}


def _build_program():
    nc = bacc.Bacc("TRN2", target_bir_lowering=False, debug=False,
                   num_devices=NCORES)

    x_in = nc.dram_tensor("xin", [T, 2 * D, G], F16, kind="ExternalInput")
    w_x0 = nc.dram_tensor("wx0", [128, 128], F16, kind="ExternalInput")
    w_h0 = nc.dram_tensor("wh0", [128, 128], F16, kind="ExternalInput")
    w_a1 = nc.dram_tensor("wa1", [128, 128], F16, kind="ExternalInput")
    w_b1 = nc.dram_tensor("wb1", [128, 128], F16, kind="ExternalInput")
    w_o = nc.dram_tensor("wo", [128, 1], F16, kind="ExternalInput")
    b_in = nc.dram_tensor("bias", [128, 4], F32, kind="ExternalInput")
    out_d = nc.dram_tensor("out", [2, G], F16, kind="ExternalOutput")

    mm = nc.tensor.matmul
    TANH = mybir.ActivationFunctionType.Tanh
    COPY = mybir.ActivationFunctionType.Copy
    MULT = mybir.AluOpType.mult
    ADD = mybir.AluOpType.add

    with tile.TileContext(nc) as tc:
        with (
            tc.tile_pool(name="const", bufs=1) as const,
            tc.tile_pool(name="state", bufs=1) as state,

            tc.tile_pool(name="act", bufs=4) as actp,
            tc.tile_pool(name="ps", bufs=2, space="PSUM") as psp,
            tc.tile_pool(name="osb", bufs=1) as osbp,
        ):
            wx0 = const.tile([128, 128], F16, tag="wx0")
            wh0 = const.tile([128, 128], F16, tag="wh0")
            wa1 = const.tile([128, 128], F16, tag="wa1")
            wb1 = const.tile([128, 128], F16, tag="wb1")
            wo = const.tile([128, 1], F16, tag="wo")
            bia = const.tile([128, 4], F32, tag="bias")
            nc.sync.dma_start(wx0, w_x0[:, :])
            nc.sync.dma_start(wh0, w_h0[:, :])
            nc.sync.dma_start(wa1, w_a1[:, :])
            nc.sync.dma_start(wb1, w_b1[:, :])
            nc.sync.dma_start(wo, w_o[:, :])
            nc.sync.dma_start(bia, b_in[:, :])

            # states: S[l][g]; g=0 state rows 0:64 / scratch 64:128, g=1 mirrored
            S = [
                [state.tile([128, G], F16, tag=f"s{l}{g}", name=f"s{l}{g}") for g in (0, 1)]
                for l in (0, 1)
            ]
            for l in (0, 1):
                for g in (0, 1):
                    nc.vector.memset(S[l][g][:, :], 0.0)
            XT = [
                state.tile([128, G], F16, tag=f"xt{i}", name=f"xt{i}")
                for i in (0, 1)
            ]
            nc.vector.memset(XT[0][:, :], 0.0)
            nc.vector.memset(XT[1][:, :], 0.0)
            R = [
                [state.tile([128, G], F16, tag=f"r{l}{g}", name=f"r{l}{g}") for g in (0, 1)]
                for l in (0, 1)
            ]

            for t in range(T):
                xt = XT[t % 2]
                nc.sync.dma_start(xt[0:2, :], x_in[t, 0:2, :])
                nc.sync.dma_start(xt[64:66, :], x_in[t, 2:4, :])

                for l in (0, 1):
                    for hf in (0, 1):
                        sl = slice(hf * HALF, (hf + 1) * HALF)
                        ps = [psp.tile([128, HALF], F32, tag="ps", name="ps") for _ in (0, 1)]
                        # interleave G0/G1 matmuls -> different PE row groups
                        # overlap in the array
                        for k in range(NMM):
                            pc = slice(k * 512, (k + 1) * 512)
                            scol = slice(hf * HALF + k * 512, hf * HALF + (k + 1) * 512)
                            for g in (0, 1):
                                r0 = 64 * g
                                if l == 0:
                                    mm(
                                        ps[g][:, pc],
                                        wx0[r0 : r0 + 64, :],
                                        xt[r0 : r0 + 64, scol],
                                        start=True,
                                        stop=False,
                                    )
                                else:
                                    rs = slice(r0, r0 + 64)
                                    mm(
                                        ps[g][:, pc],
                                        wa1[rs, :],
                                        S[0][g][rs, scol],
                                        start=True,
                                        stop=False,
                                    )
                            for g in (0, 1):
                                r0 = 64 * g
                                rs = slice(r0, r0 + 64)
                                if l == 0:
                                    mm(
                                        ps[g][:, pc],
                                        wh0[rs, :],
                                        S[0][g][rs, scol],
                                        start=False,
                                        stop=True,
                                    )
                                else:
                                    mm(
                                        ps[g][:, pc],
                                        wb1[rs, :],
                                        S[1][g][rs, scol],
                                        start=False,
                                        stop=True,
                                    )
                        for g in (0, 1):
                            st = S[l][g]
                            a = actp.tile([128, HALF], F16, tag="act")
                            nc.scalar.activation(
                                a[:, :], ps[g][:, :], TANH, bias=bia[:, l * 2 + g : l * 2 + g + 1]
                            )
                            if g == 0:
                                tau, hrow, srow = a[0:64, :], slice(0, 64), slice(64, 128)
                            else:
                                tau, hrow, srow = a[64:128, :], slice(64, 128), slice(0, 64)
                            # b-gate into the scratch half of the state tensor
                            nc.vector.tensor_scalar(
                                st[srow, sl], tau, -0.5, 0.5, MULT, ADD
                            )
                            # tau -> a-gate in place
                            nc.vector.tensor_scalar(tau, tau, 0.5, 0.5, MULT, ADD)
                            # [a;c] (*) [h;b]  (G1: [c;a] (*) [b;h])
                            nc.vector.tensor_mul(st[:, sl], a[:, :], st[:, sl])
                    # state halves sum: h_new = a*h + b*c
                    for g in (0, 1):
                        st = S[l][g]
                        dst = slice(0, 64) if g == 0 else slice(64, 128)
                        srows = slice(64, 128) if g == 0 else slice(0, 64)
                        if USE_ACCUM_DMA:
                            nc.gpsimd.dma_start(
                                st[dst, :], st[srows, :], accum_op=ADD
                            )
                        else:
                            # realign the other product half to the same
                            # partition base via HWDGE DMA, then same-base add
                            rr = R[l][g]
                            nc.sync.dma_start(rr[dst, :], st[srows, :])
                            nc.vector.tensor_add(st[dst, :], st[dst, :], rr[dst, :])

            # output projection: out = h1 @ Wo  (bo added on host)
            osb = osbp.tile([128, G], F16, tag="osb")
            for hf in (0, 1):
                ps = [psp.tile([128, HALF], F32, tag="ps", name="ps") for _ in (0, 1)]
                for k in range(NMM):
                    pc = slice(k * 512, (k + 1) * 512)
                    scol = slice(hf * HALF + k * 512, hf * HALF + (k + 1) * 512)
                    mm(ps[0][0:1, pc], wo[0:64, :], S[1][0][0:64, scol],
                       start=True, stop=True)
                    mm(ps[1][64:65, pc], wo[64:128, :], S[1][1][64:128, scol],
                       start=True, stop=True)
                sl = slice(hf * HALF, (hf + 1) * HALF)
                nc.scalar.activation(osb[0:1, sl], ps[0][0:1, :], COPY)
                nc.scalar.activation(osb[64:65, sl], ps[1][64:65, :], COPY)
            nc.sync.dma_start(out_d[0:1, :], osb[0:1, :])
            nc.sync.dma_start(out_d[1:2, :], osb[64:65, :])

    nc.compile()
    return nc


def _fold_weights(Wu0, Wc0, Wu1, Wc1, Wo, bu0, bc0, bu1, bc1):
    """Host-side folding into the device layout (fp32 -> bf16)."""
    bf = np.float16

    def cell_w(Wu, Wc):  # [K, 64] x2 -> G0 [K,128] = [0.5*Wu | Wc], G1 swapped
        g0 = np.concatenate([0.5 * Wu, Wc], axis=1)
        g1 = np.concatenate([Wc, 0.5 * Wu], axis=1)
        return g0, g1

    def pack(g0, g1, k):
        w = np.zeros((128, 128), np.float32)
        w[0:k] = g0
        w[64 : 64 + k] = g1
        return w.astype(bf)

    wx0 = pack(*cell_w(Wu0[0:2], Wc0[0:2]), 2)
    wh0 = pack(*cell_w(Wu0[2:66], Wc0[2:66]), 64)
    wa1 = pack(*cell_w(Wu1[0:64], Wc1[0:64]), 64)
    wb1 = pack(*cell_w(Wu1[64:128], Wc1[64:128]), 64)
    wo = np.zeros((128, 1), np.float32)
    wo[0:64] = Wo
    wo[64:128] = Wo
    wo = wo.astype(bf)
    bias = np.zeros((128, 4), np.float32)
    for l, (bu, bc) in enumerate([(bu0, bc0), (bu1, bc1)]):
        bias[0:64, 2 * l + 0] = 0.5 * bu
        bias[64:128, 2 * l + 0] = bc
        bias[0:64, 2 * l + 1] = bc
        bias[64:128, 2 * l + 1] = 0.5 * bu
    return dict(wx0=wx0, wh0=wh0, wa1=wa1, wb1=wb1, wo=wo, bias=bias)


def _get_runner():
    """Build the Bass program once and wrap it in a persistent jitted
    shard_map callable (the hoisted equivalent of what bass_utils.
    run_bass_kernel_spmd -> bass2jax.run_bass_via_pjrt builds per call).

    run_bass_kernel_spmd constructs a fresh jax.jit closure on every
    invocation, which forces a full retrace + NEFF recompile (~450ms) per
    call; hoisting the jit out makes warm calls hit the C++ pjit fast path.
    """
    if "runner" in _CACHE:
        return _CACHE["runner"]

    import jax
    from jax.sharding import Mesh, PartitionSpec, NamedSharding
    from jax.experimental.shard_map import shard_map
    from concourse import bass2jax as b2j

    b2j.install_neuronx_cc_hook()
    nc = _build_program()
    assert nc.dbg_addr is None
    pname = nc.partition_id_tensor.name if nc.partition_id_tensor else None

    in_names, out_names, out_avals, zero_outs = [], [], [], []
    for alloc in nc.m.functions[0].allocations:
        if not isinstance(alloc, mybir.MemoryLocationSet):
            continue
        name = alloc.memorylocations[0].name
        if alloc.kind == "ExternalInput":
            if name != pname:
                in_names.append(name)
        elif alloc.kind == "ExternalOutput":
            out_names.append(name)
            shape = tuple(alloc.tensor_shape)
            dtype = mybir.dt.np(alloc.dtype)
            out_avals.append(jax.core.ShapedArray(shape, dtype))
            zero_outs.append(np.zeros((NCORES * shape[0], *shape[1:]), dtype))
    n_params = len(in_names)
    n_outs = len(out_names)
    all_names = tuple(in_names + out_names + ([pname] if pname else []))

    def _body(*args):
        operands = list(args)
        if pname is not None:
            operands.append(b2j.partition_id_tensor())
        return tuple(
            b2j._bass_exec_p.bind(
                *operands,
                out_avals=tuple(out_avals),
                in_names=all_names,
                out_names=tuple(out_names),
                lowering_input_output_aliases=(),
                sim_require_finite=True,
                sim_require_nnan=True,
                nc=nc,
            )
        )

    devices = jax.devices()[:NCORES]
    mesh = Mesh(np.asarray(devices), ("core",))
    specs = (PartitionSpec("core"),) * (n_params + n_outs)
    # No donate_argnums: the kernel writes every element of `out`, so the
    # zero-init aliasing run_bass_via_pjrt sets up is unnecessary; without
    # donation the zero operands can stay device-resident across calls.
    fn = jax.jit(
        shard_map(_body, mesh=mesh, in_specs=specs, out_specs=specs[:n_outs],
                  check_rep=False),
        keep_unused=True,
    )
    sharding = NamedSharding(mesh, PartitionSpec("core"))
    runner = dict(
        fn=fn, in_names=in_names,
        zero_dev=[jax.device_put(z, sharding) for z in zero_outs],
        sharding=sharding, jax=jax,
    )
    _CACHE["runner"] = runner
    return runner


def _dev_weights(inputs, runner):
    """Fold weights and park them on-device once (they are model constants);
    keyed by content so a different parameter set still recomputes."""
    names = ("Wu0", "Wc0", "Wu1", "Wc1", "Wo", "bu0", "bc0", "bu1", "bc1")
    ws = [np.asarray(inputs[n], np.float32) for n in names]
    key = hash(tuple(w.tobytes() for w in ws))
    if _CACHE.get("wkey") != key:
        folded = _fold_weights(*ws)
        jax_ = runner["jax"]
        _CACHE["wdev"] = {
            k: jax_.device_put(
                np.concatenate([v] * NCORES, axis=0), runner["sharding"]
            )
            for k, v in folded.items()
        }
        _CACHE["wkey"] = key
    return _CACHE["wdev"]


def kernel(**inputs):
    runner = _get_runner()
    wdev = _dev_weights(inputs, runner)
    bo = np.asarray(inputs["bo"], np.float32)

    # token order: flat (b, n); core c owns tokens [c*8192, (c+1)*8192)
    # per-core xin [T, 2D, G] -> global concat on axis 0: [NCORES*T, 2D, G].
    # xg[c*T+t, p*D+d, q*N+n] = x[4c+2p+q, t, n, d]; one strided copyto pass
    # into a persistent buffer (avoids per-call page-fault cost).
    x = np.asarray(inputs["x"], np.float32)
    if "xg" not in _CACHE:
        _CACHE["xg"] = np.empty((NCORES * T, 2 * D, G), np.float16)
    xg = _CACHE["xg"]
    np.copyto(
        xg.reshape(NCORES, T, 2, D, 2, N),
        x.reshape(NCORES, 2, 2, T, N, D).transpose(0, 3, 1, 5, 2, 4),
    )

    args = [xg if n == "xin" else wdev[n] for n in runner["in_names"]]
    out_arrs = runner["fn"](*args, *runner["zero_dev"])
    out = np.asarray(out_arrs[0])  # [NCORES*2, G] f16, one blocking fetch
    return (out.reshape(-1).astype(np.float32).reshape(B, N, O) + bo)


if __name__ == "__main__":
    rng = np.random.default_rng(0)
    fake = {
        "x": rng.standard_normal((B, T, N, D), dtype=np.float32),
        "supports": rng.random((2, N, N), dtype=np.float32),
        "Wo": (rng.standard_normal((H, O)) * 0.02).astype(np.float32),
        "bo": np.zeros((O,), np.float32),
    }
    for l in range(2):
        din = (D if l == 0 else H) + H
        for g in ("r", "u", "c"):
            fake[f"W{g}{l}"] = (rng.standard_normal((din, H)) * 0.02).astype(np.float32)
            fake[f"b{g}{l}"] = np.zeros((H,), np.float32)
        fake[f"Wd{l}"] = (rng.standard_normal((2, H, H)) * 0.02).astype(np.float32)
        fake[f"bd{l}"] = np.zeros((2, H), np.float32)
    print(kernel(**fake).shape)

